# revision 1
# baseline (speedup 1.0000x reference)
"""GPT-2 attention (B=16, S=1024, E=768, H=12, D=64) on 8 TRN2 NeuronCores.

Sharding: data-parallel over batch — each core processes B_LOC=2 batch
elements with fully replicated weights. No collectives.

Per-core pipeline (per batch element):
  A. x [1024,768] -> x^T in SBUF via PE transposes (48 [128,128] tiles)
  B. v = x @ W_attn[:,1536:2304] + b  (seq-major, heads interleaved with a
     fused ones-column per head for softmax denominators)
  C. per head-pair t (q/k feature tile):
     C1. q^T, k^T = (x @ W)^T via lhsT=W chunks, rhs=x^T  (feature-major)
     C2. per head: scores^T bands (lhsT=k^T slice, rhs=q^T) -> Exp(s/8) from
         PSUM -> causal tri-mask on diagonal block -> attn@v with fused
         colsum (M=65) accumulating over bands -> reciprocal -> K=1
         outer-product broadcast -> normalize into attn_out^T
  D. out = attn_out @ W_proj + b_proj (bias via K=1 outer product into PSUM)

All matmuls run in float32r (full-rate PE, ~1.6e-4 rel err measured on HW).
"""

import sys

sys.path.insert(0, "/opt/trn_rl_repo")

from contextlib import ExitStack

import numpy as np

import concourse.bass as bass
import concourse.mybir as mybir
import concourse.tile as tile
from concourse.masks import make_identity

F32 = mybir.dt.float32
F32R = mybir.dt.float32r
BF16 = mybir.dt.bfloat16
AF = mybir.ActivationFunctionType

B, S, E = 16, 1024, 768
H, D = 12, 64
NCORES = 8
B_LOC = B // NCORES          # 2 batch elements per core
KC = E // 128                # 6 contraction chunks
ST = S // 128                # 8 seq tiles
PAIRS = H // 2               # 6 head pairs (2 heads per 128-row feature tile)


def emit(tc, outs, ins):
    nc = tc.nc
    x, wa, ba, wp, bp = (ins["hidden_states"], ins["W_attn"], ins["b_attn"],
                         ins["W_proj"], ins["b_proj"])
    out = outs["out"]
    # weights/biases are consumed as f32r matmul operands; the real build()
    # declares them f32r in DRAM, the sim harness allocates fp32 — reinterpret
    wa = wa if wa.dtype == F32R else wa.bitcast(F32R)
    ba = ba if ba.dtype == F32R else ba.bitcast(F32R)
    wp = wp if wp.dtype == F32R else wp.bitcast(F32R)
    bp = bp if bp.dtype == F32R else bp.bitcast(F32R)

    ctx = ExitStack()
    with ctx:
        wpool = ctx.enter_context(tc.tile_pool(name="wpool", bufs=1))
        work = ctx.enter_context(tc.tile_pool(name="work", bufs=1))
        ps = ctx.enter_context(tc.tile_pool(name="ps", bufs=2, space="PSUM"))

        # ---------- persistent weights (f32r via in-place rounding copy)
        wa_r = []
        for k in range(KC):
            w = wpool.tile([128, 3 * E], F32R, tag=f"wa{k}", name=f"wa{k}")
            nc.sync.dma_start(w, wa[k * 128:(k + 1) * 128, :])
            wa_r.append(w)
        wp_r = []
        for k in range(KC):
            w = wpool.tile([128, E], F32R, tag=f"wp{k}", name=f"wp{k}")
            nc.sync.dma_start(w, wp[k * 128:(k + 1) * 128, :])
            wp_r.append(w)

        # q/k bias, feature-major [128, 12]: (p, m) = b_attn[m*128 + p]
        ba_qk = wpool.tile([128, 2 * KC], F32)
        nc.sync.dma_start(ba_qk.bitcast(F32R),
                          ba[0:2 * E].rearrange("(m p) -> p m", p=128))
        # v bias and proj bias as rows (outer-product rhs), f32r
        ba_v = wpool.tile([1, E], F32R)
        nc.sync.dma_start(ba_v, ba[2 * E:3 * E].unsqueeze(0))
        bp_r = wpool.tile([1, E], F32R)
        nc.sync.dma_start(bp_r, bp.unsqueeze(0))

        identity = wpool.tile([128, 128], F32)
        make_identity(nc, identity)

        ones_col32 = wpool.tile([128, 1], F32)
        nc.vector.memset(ones_col32, 1.0)
        ones_row32 = wpool.tile([1, 128], F32)
        nc.vector.memset(ones_row32, 1.0)
        ones_row = wpool.tile([1, 128], F32R)
        nc.vector.tensor_copy(ones_row, ones_row32)

        # pre-broadcast v/proj biases to [128, E] so bias-add fuses into the
        # PSUM->SBUF copy on DVE (replaces per-tile K=1 outer products on PE)
        biasv_bc = wpool.tile([128, E], F32)
        biasp_bc = wpool.tile([128, E], F32)
        for bc_dst, brow in ((biasv_bc, ba_v), (biasp_bc, bp_r)):
            for n0, nw in ((0, 512), (512, 256)):
                bps = ps.tile([128, 512], F32, tag="tr", name=f"bbc{n0}_{brow.name}")
                nc.tensor.matmul(bps[:, 0:nw], ones_row, brow[0:1, n0:n0 + nw],
                                 start=True, stop=True)
                nc.scalar.activation(bc_dst[:, n0:n0 + nw], bps[:, 0:nw], AF.Copy)

        # causal tri-mask [128,128]: mask[r,c] = 1 if c >= r else 0
        trimask32 = wpool.tile([128, 128], F32)
        nc.gpsimd.memset(trimask32, 1.0)
        nc.gpsimd.affine_select(
            out=trimask32, in_=trimask32, compare_op=mybir.AluOpType.is_ge,
            fill=0.0, base=0, pattern=[[1, 128]], channel_multiplier=-1,
        )
        trimask = wpool.tile([128, 128], BF16)
        nc.vector.tensor_copy(trimask, trimask32)

        for b in range(B_LOC):
            # ---------- A: x^T via PE transposes
            xT = []
            for k in range(KC):
                t_ = work.tile([128, S], F32R, tag=f"xt{k}", name=f"xT{k}_{b}")
                xT.append(t_)
            for k in range(KC):
                for st in range(ST):
                    xin = work.tile([128, 128], F32, tag="xin", bufs=3,
                                    name=f"xin{b}_{k}_{st}")
                    nc.sync.dma_start(
                        xin, x[b, st * 128:(st + 1) * 128, k * 128:(k + 1) * 128])
                    tr_ps = ps.tile([128, 128], F32, tag="tr", name=f"tr{b}_{k}_{st}")
                    nc.tensor.transpose(tr_ps, xin, identity)
                    nc.scalar.activation(
                        xT[k][:, st * 128:(st + 1) * 128], tr_ps, AF.Copy)

            # ---------- B: v (seq-major, 12 heads x [64 cols + ones col])
            v_r = []
            for st in range(ST):
                vt = work.tile([128, H, D + 1], BF16, tag=f"v{st}", name=f"v{st}_{b}")
                v_r.append(vt)
                nc.vector.tensor_copy(
                    vt[:, :, D:D + 1], ones_col32.broadcast_to((128, H, 1)))
                for n0, nw in ((0, 512), (512, 256)):
                    acc = ps.tile([128, 512], F32, tag="acc", name=f"vacc{b}_{st}_{n0}")
                    for k in range(KC):
                        nc.tensor.matmul(
                            acc[:, 0:nw],
                            xT[k][:, st * 128:(st + 1) * 128],
                            wa_r[k][:, 2 * E + n0:2 * E + n0 + nw],
                            start=(k == 0), stop=(k == KC - 1))
                    nc.vector.tensor_add(
                        vt[:, n0 // D:(n0 + nw) // D, 0:D],
                        acc[:, 0:nw].rearrange("p (h d) -> p h d", d=D),
                        biasv_bc[:, n0:n0 + nw].rearrange("p (h d) -> p h d", d=D))

            # ---------- C: head pairs
            aoT = []
            for t in range(PAIRS):
                at = work.tile([128, S], F32R, tag=f"ao{t}", name=f"aoT{t}_{b}")
                aoT.append(at)
            for t in range(PAIRS):
                # C1: q^T, k^T feature tiles for this pair
                q_r = work.tile([128, S], F32R, tag="qt", bufs=2, name=f"q{t}_{b}")
                k_r = work.tile([128, S], F32R, tag="kt", bufs=2, name=f"k{t}_{b}")
                for dst, m in ((q_r, t), (k_r, KC + t)):
                    for c0 in (0, 512):
                        acc = ps.tile([128, 512], F32, tag="acc",
                                      name=f"qk{b}_{m}_{c0}")
                        for k in range(KC):
                            nc.tensor.matmul(
                                acc, wa_r[k][:, m * 128:(m + 1) * 128],
                                xT[k][:, c0:c0 + 512],
                                start=(k == 0), stop=(k == KC - 1))
                        nc.scalar.activation(
                            dst[:, c0:c0 + 512], acc, AF.Identity,
                            bias=ba_qk[:, m:m + 1])

                # C2: the two heads of this pair
                for hh in range(2):
                    h = 2 * t + hh
                    po = hh * 64
                    av0 = ps.tile([65, 512], F32, tag="av", name=f"av0_{b}_{h}")
                    av1 = ps.tile([65, 512], F32, tag="av", name=f"av1_{b}_{h}")
                    av = (av0, av1)
                    for kb in range(ST):
                        k0 = kb * 128
                        ncols = S - k0
                        exp_t = work.tile([128, ncols], BF16, tag="exp", bufs=4,
                                          name=f"exp{b}_{h}_{kb}")
                        w = S - k0
                        if w % 512 == 128 and w > 128:
                            chunks = [(k0 + o, cw) for o, cw in
                                      zip((0, w - 640, w - 256), (512,) * ((w - 640) // 512) + (384, 256))]
                            chunks = []
                            o = k0
                            rem = w
                            while rem > 640:
                                chunks.append((o, 512)); o += 512; rem -= 512
                            chunks += [(o, 384), (o + 384, 256)]
                        else:
                            chunks = []
                            o = k0
                            while o < S:
                                cw = min(512, S - o)
                                chunks.append((o, cw)); o += cw
                        for c0, cw in chunks:
                            sc = ps.tile([128, 512], F32, tag="sc",
                                         name=f"sc{b}_{h}_{kb}_{c0}")
                            nc.tensor.matmul(
                                sc[:, 0:cw],
                                k_r[po:po + 64, k0:k0 + 128],
                                q_r[po:po + 64, c0:c0 + cw],
                                start=True, stop=True)
                            nc.scalar.activation(
                                exp_t[:, c0 - k0:c0 - k0 + cw], sc[:, 0:cw],
                                AF.Exp, scale=0.125)
                        # causal mask on the diagonal block
                        nc.vector.tensor_mul(
                            exp_t[:, 0:128], exp_t[:, 0:128], trimask)
                        # attn@v contributions of this band
                        for ci, s0 in enumerate((0, 512)):
                            if k0 < s0 + 512:
                                lo = max(s0, k0)
                                last_kb = 3 if ci == 0 else 7
                                nc.tensor.matmul(
                                    av[ci][:, lo - s0:512],
                                    v_r[kb][:, h, :],
                                    exp_t[:, lo - k0:s0 + 512 - k0],
                                    start=(kb == 0), stop=(kb == last_kb))
                    # copy denominators + unnormalized attn_out^T out of PSUM
                    # (frees the av bank before the slow reciprocal runs)
                    for ci, s0 in enumerate((0, 512)):
                        srow = work.tile([1, 512], F32, tag="srow", bufs=2,
                                         name=f"srow_{b}_{h}_{ci}")
                        nc.scalar.activation(srow, av[ci][64:65, :], AF.Copy)
                        dst = aoT[t][po:po + 64, s0:s0 + 512]
                        nc.vector.tensor_copy(dst, av[ci][0:64, :])
                        rrow = work.tile([1, 512], F32R, tag="rrow", bufs=2,
                                         name=f"rrow_{b}_{h}_{ci}")
                        with nc.allow_low_precision(reason="denom f32r"):
                            nc.vector.reciprocal(rrow, srow)
                        bc = ps.tile([128, 512], F32, tag="tr",
                                     name=f"bc{b}_{h}_{ci}")
                        nc.tensor.matmul(bc, ones_row, rrow, start=True, stop=True)
                        nc.vector.tensor_mul(dst, dst, bc[po:po + 64, :])

            # ---------- D: proj
            for st in range(ST):
                outt = work.tile([128, E], F32, tag="outt", bufs=2,
                                 name=f"outt{b}_{st}")
                for n0, nw in ((0, 512), (512, 256)):
                    acc = ps.tile([128, 512], F32, tag="acc",
                                  name=f"pacc{b}_{st}_{n0}")
                    for k in range(KC):
                        nc.tensor.matmul(
                            acc[:, 0:nw],
                            aoT[k][:, st * 128:(st + 1) * 128],
                            wp_r[k][:, n0:n0 + nw],
                            start=(k == 0), stop=(k == KC - 1))
                    nc.vector.tensor_add(outt[:, n0:n0 + nw], acc[:, 0:nw],
                                         biasp_bc[:, n0:n0 + nw])
                nc.sync.dma_start(out[b, st * 128:(st + 1) * 128, :], outt)


def build():
    from concourse import bacc

    nc = bacc.Bacc("TRN2", target_bir_lowering=False, debug=False)
    ins = {
        "hidden_states": nc.dram_tensor(
            "hidden_states", [B_LOC, S, E], F32, kind="ExternalInput").ap(),
        "W_attn": nc.dram_tensor("W_attn", [E, 3 * E], F32R,
                                 kind="ExternalInput").ap(),
        "b_attn": nc.dram_tensor("b_attn", [3 * E], F32R,
                                 kind="ExternalInput").ap(),
        "W_proj": nc.dram_tensor("W_proj", [E, E], F32R,
                                 kind="ExternalInput").ap(),
        "b_proj": nc.dram_tensor("b_proj", [E], F32R, kind="ExternalInput").ap(),
    }
    outs = {
        "out": nc.dram_tensor("out", [B_LOC, S, E], F32,
                              kind="ExternalOutput").ap(),
    }
    with tile.TileContext(nc) as tc:
        emit(tc, outs, ins)
    nc.compile()
    return nc


_CACHED_NC = None


def kernel(hidden_states, W_attn, b_attn, W_proj, b_proj, trace=False):
    global _CACHED_NC
    from concourse.bass_utils import run_bass_kernel_spmd

    if _CACHED_NC is None:
        _CACHED_NC = build()
    nc = _CACHED_NC

    hidden_states = np.ascontiguousarray(hidden_states, dtype=np.float32)
    W_attn = np.ascontiguousarray(W_attn, dtype=np.float32)
    b_attn = np.ascontiguousarray(b_attn, dtype=np.float32)
    W_proj = np.ascontiguousarray(W_proj, dtype=np.float32)
    b_proj = np.ascontiguousarray(b_proj, dtype=np.float32)

    in_maps = []
    for c in range(NCORES):
        in_maps.append({
            "hidden_states": hidden_states[c * B_LOC:(c + 1) * B_LOC],
            "W_attn": W_attn, "b_attn": b_attn,
            "W_proj": W_proj, "b_proj": b_proj,
        })
    res = run_bass_kernel_spmd(nc, in_maps, core_ids=list(range(NCORES)),
                               trace=trace)
    out = np.concatenate([res.results[c]["out"] for c in range(NCORES)], axis=0)
    kernel.last_result = res
    return out



# revision 4
# speedup vs baseline: 1.5186x; 1.5186x over previous
"""GPT-2 attention (B=16, S=1024, E=768, H=12, D=64) on 8 TRN2 NeuronCores.

Sharding: data-parallel over batch — each core processes B_LOC=2 batch
elements with fully replicated weights. No collectives.

v2 design (vs v1 baseline):
  * all matmul operands fp16 (1 cycle/row at any moving size; PSUM accum f32)
  * attn@v emitted SEQ-major: out[q, d] with a fused ones-column in v giving
    the softmax denominator as PSUM COLUMN 64 -> normalize is a per-partition
    reciprocal [128,1] + tensor_scalar, killing v1's [1,512] reciprocals
    (3.3us each), PE broadcast matmuls and [64,512] normalize muls
  * scores bands accumulate into 2-bank PSUM tiles; ONE wide exp per band
  * causal tri-mask multiply on the (otherwise idle) GpSimd engine
  * x^T via f32r PE transposes (1.5 c/row), copied out as fp16
  * attn out transposed back (fp16 transposes, 1 c/row) for the proj matmul
"""

import sys

sys.path.insert(0, "/opt/trn_rl_repo")

from contextlib import ExitStack

import numpy as np

import concourse.bass as bass
import concourse.mybir as mybir
import concourse.tile as tile
from concourse.masks import make_identity

F32 = mybir.dt.float32
F32R = mybir.dt.float32r
F16 = mybir.dt.float16
AF = mybir.ActivationFunctionType
ALU = mybir.AluOpType

B, S, E = 16, 1024, 768
H, D = 12, 64
NCORES = 8
B_LOC = B // NCORES          # 2 batch elements per core
KC = E // 128                # 6 contraction chunks
ST = S // 128                # 8 seq tiles
PAIRS = H // 2               # 6 head pairs (2 heads per 128-row feature tile)

# band kb covers k in [kb*128, (kb+1)*128), q in [kb*128, S): width 1024-kb*128
BAND_W = [S - kb * 128 for kb in range(ST)]
BAND_OFF = [sum(BAND_W[:kb]) for kb in range(ST)]
EXP_COLS = sum(BAND_W)       # 4608


def emit(tc, outs, ins):
    nc = tc.nc
    x, wa, ba, wp, bp = (ins["hidden_states"], ins["W_attn"], ins["b_attn"],
                         ins["W_proj"], ins["b_proj"])
    out = outs["out"]
    xr = x.bitcast(F32R)
    ba_r = ba.bitcast(F32R)
    bp_r_d = bp.bitcast(F32R)

    ctx = ExitStack()
    with ctx:
        wpool = ctx.enter_context(tc.tile_pool(name="wpool", bufs=1))
        work = ctx.enter_context(tc.tile_pool(name="work", bufs=1))
        ps = ctx.enter_context(tc.tile_pool(name="ps", bufs=1, space="PSUM"))

        # ---------- weights: DMA f32 staging -> fp16 SBUF
        wa_h = []
        for k in range(KC):
            w = wpool.tile([128, 3 * E], F16, tag=f"wa{k}", name=f"wa{k}")
            stage = work.tile([128, 3 * E], F32, tag="wstage", bufs=2,
                              name=f"was{k}")
            nc.sync.dma_start(stage, wa[k * 128:(k + 1) * 128, :])
            nc.scalar.activation(w, stage, AF.Copy)
            wa_h.append(w)
        wp_h = []
        for k in range(KC):
            w = wpool.tile([128, E], F16, tag=f"wp{k}", name=f"wp{k}")
            stage = work.tile([128, E], F32, tag="wpstage", bufs=2,
                              name=f"wps{k}")
            nc.sync.dma_start(stage, wp[k * 128:(k + 1) * 128, :])
            nc.vector.tensor_copy(w, stage)
            wp_h.append(w)

        # q/k bias, feature-major [128, 12]: (p, m) = b_attn[m*128 + p]
        ba_qk = wpool.tile([128, 2 * KC], F32)
        nc.sync.dma_start(ba_qk.bitcast(F32R),
                          ba_r[0:2 * E].rearrange("(m p) -> p m", p=128))
        # v bias and proj bias as rows (outer-product rhs), f32r
        ba_v = wpool.tile([1, E], F32R)
        nc.sync.dma_start(ba_v, ba_r[2 * E:3 * E].unsqueeze(0))
        bp_row = wpool.tile([1, E], F32R)
        nc.sync.dma_start(bp_row, bp_r_d.unsqueeze(0))

        identity = wpool.tile([128, 128], F32)
        make_identity(nc, identity)
        ident_h = wpool.tile([128, 128], F16)
        nc.vector.tensor_copy(ident_h, identity)
        ident_r = wpool.tile([128, 128], F32R)
        nc.vector.tensor_copy(ident_r, identity)

        ones_row32 = wpool.tile([1, 128], F32)
        nc.vector.memset(ones_row32, 1.0)
        ones_row = wpool.tile([1, 128], F32R)
        nc.vector.tensor_copy(ones_row, ones_row32)

        # broadcast v/proj biases to [128, E] (bias-add fuses into PSUM->SBUF)
        biasv_bc = wpool.tile([128, E], F32)
        biasp_bc = wpool.tile([128, E], F32)
        for bc_dst, brow in ((biasv_bc, ba_v), (biasp_bc, bp_row)):
            for n0, nw in ((0, 512), (512, 256)):
                bps = ps.tile([128, 512], F32, tag="avtr", bufs=2,
                              name=f"bbc{n0}_{brow.name}")
                nc.tensor.matmul(bps[:, 0:nw], ones_row, brow[0:1, n0:n0 + nw],
                                 start=True, stop=True)
                nc.scalar.activation(bc_dst[:, n0:n0 + nw], bps[:, 0:nw],
                                     AF.Copy)

        # causal tri-mask [128,128] fp16: mask[r,c] = 1 if c >= r else 0
        trimask32 = wpool.tile([128, 128], F32)
        nc.gpsimd.memset(trimask32, 1.0)
        nc.gpsimd.affine_select(
            out=trimask32, in_=trimask32, compare_op=ALU.is_ge,
            fill=0.0, base=0, pattern=[[1, 128]], channel_multiplier=-1,
        )
        trimask = wpool.tile([128, 128], F16)
        nc.vector.tensor_copy(trimask, trimask32)

        for b in range(B_LOC):
            # ---------- A: x^T (fp16) via f32r PE transposes
            xT = []
            for k in range(KC):
                t_ = work.tile([128, S], F16, tag=f"xt{k}", bufs=2,
                               name=f"xT{k}_{b}")
                xT.append(t_)
            for st in range(ST):
                xin = work.tile([128, E], F32R, tag="xin", bufs=3,
                                name=f"xin{b}_{st}")
                nc.sync.dma_start(xin, xr[b, st * 128:(st + 1) * 128, :])
                for k in range(KC):
                    tr_ps = ps.tile([128, 128], F32R, tag="avtr", bufs=2,
                                    name=f"tr{b}_{k}_{st}")
                    nc.tensor.transpose(tr_ps, xin[:, k * 128:(k + 1) * 128],
                                        ident_r)
                    nc.vector.tensor_copy(
                        xT[k][:, st * 128:(st + 1) * 128], tr_ps.bitcast(F32))

            # ---------- B: v seq-major [128, 12, 65] fp16 with ones col
            v_r = []
            for st in range(ST):
                vt = work.tile([128, H, D + 1], F16, tag=f"v{st}",
                               name=f"v{st}_{b}")
                v_r.append(vt)
                nc.vector.memset(vt[:, :, D:D + 1], 1.0)
                acc = ps.tile([128, E], F32, tag="acc", bufs=1,
                              name=f"vacc{b}_{st}")
                for n0, nw in ((0, 512), (512, 256)):
                    for k in range(KC):
                        nc.tensor.matmul(
                            acc[:, n0:n0 + nw],
                            xT[k][:, st * 128:(st + 1) * 128],
                            wa_h[k][:, 2 * E + n0:2 * E + n0 + nw],
                            start=(k == 0), stop=(k == KC - 1))
                nc.vector.tensor_add(
                    vt[:, :, 0:D],
                    acc.rearrange("p (h d) -> p h d", d=D),
                    biasv_bc.rearrange("p (h d) -> p h d", d=D))

            # ---------- C: head pairs
            ao = []
            for st in range(ST):
                at = work.tile([128, E], F16, tag=f"ao{st}", name=f"ao{st}_{b}")
                ao.append(at)
            for t in range(PAIRS):
                # C1: q^T, k^T feature tiles for this pair (fp16, biased)
                q_r = work.tile([128, S], F16, tag="qt", bufs=2, name=f"q{t}_{b}")
                k_r = work.tile([128, S], F16, tag="kt", bufs=2, name=f"k{t}_{b}")
                for dst, m in ((q_r, t), (k_r, KC + t)):
                    qkacc = ps.tile([128, S], F32, tag="acc", bufs=1,
                                    name=f"qk{b}_{m}")
                    for c0 in (0, 512):
                        for k in range(KC):
                            nc.tensor.matmul(
                                qkacc[:, c0:c0 + 512],
                                wa_h[k][:, m * 128:(m + 1) * 128],
                                xT[k][:, c0:c0 + 512],
                                start=(k == 0), stop=(k == KC - 1))
                    nc.vector.tensor_scalar(
                        dst, qkacc, ba_qk[:, m:m + 1], None, op0=ALU.add)

                # C2: the two heads of this pair
                for hh in range(2):
                    h = 2 * t + hh
                    po = hh * 64
                    exp_h = work.tile([128, EXP_COLS], F16, tag="exp", bufs=2,
                                      name=f"exp{b}_{h}")
                    for kb in range(ST):
                        k0 = kb * 128
                        w = BAND_W[kb]
                        off = BAND_OFF[kb]
                        sc = ps.tile([128, 1024], F32, tag="sc", bufs=2,
                                     name=f"sc{b}_{h}_{kb}")
                        c0 = k0
                        while c0 < S:
                            cw = min(512, S - c0)
                            nc.tensor.matmul(
                                sc[:, c0 - k0:c0 - k0 + cw],
                                k_r[po:po + 64, k0:k0 + 128],
                                q_r[po:po + 64, c0:c0 + cw],
                                start=True, stop=True)
                            c0 += cw
                        nc.scalar.activation(
                            exp_h[:, off:off + w], sc[:, 0:w],
                            AF.Exp, scale=0.125)
                        # causal mask on the diagonal block (idle GpSimd)
                        nc.gpsimd.tensor_mul(
                            exp_h[:, off:off + 128], exp_h[:, off:off + 128],
                            trimask)
                    for qt in range(ST):
                        av = ps.tile([128, D + 1], F32, tag="avtr", bufs=2,
                                     name=f"av{b}_{h}_{qt}")
                        for kb in range(qt + 1):
                            sl = BAND_OFF[kb] + (qt - kb) * 128
                            nc.tensor.matmul(
                                av, exp_h[:, sl:sl + 128],
                                v_r[kb][:, h, :],
                                start=(kb == 0), stop=(kb == qt))
                        rc = work.tile([128, 1], F32, tag="rc", bufs=4,
                                       name=f"rc{b}_{h}_{qt}")
                        nc.vector.reciprocal(rc, av[:, D:D + 1])
                        nc.vector.tensor_scalar(
                            ao[qt][:, h * D:(h + 1) * D], av[:, 0:D],
                            rc, None, op0=ALU.mult)

            # ---------- transpose attn out back to feature-major
            aoT = []
            for k in range(KC):
                at = work.tile([128, S], F16, tag=f"aot{k}", name=f"aoT{k}_{b}")
                aoT.append(at)
            for st in range(ST):
                for k in range(KC):
                    tr_ps = ps.tile([128, 128], F16, tag="avtr", bufs=2,
                                    name=f"aotr{b}_{st}_{k}")
                    nc.tensor.transpose(
                        tr_ps, ao[st][:, k * 128:(k + 1) * 128], ident_h)
                    nc.vector.tensor_copy(
                        aoT[k][:, st * 128:(st + 1) * 128], tr_ps)

            # ---------- D: proj
            for st in range(ST):
                outt = work.tile([128, E], F32, tag="outt", bufs=2,
                                 name=f"outt{b}_{st}")
                pacc = ps.tile([128, E], F32, tag="acc", bufs=1,
                               name=f"pacc{b}_{st}")
                for n0, nw in ((0, 512), (512, 256)):
                    for k in range(KC):
                        nc.tensor.matmul(
                            pacc[:, n0:n0 + nw],
                            aoT[k][:, st * 128:(st + 1) * 128],
                            wp_h[k][:, n0:n0 + nw],
                            start=(k == 0), stop=(k == KC - 1))
                nc.vector.tensor_add(outt, pacc, biasp_bc)
                nc.sync.dma_start(out[b, st * 128:(st + 1) * 128, :], outt)


def build():
    from concourse import bacc

    nc = bacc.Bacc("TRN2", target_bir_lowering=False, debug=False)
    ins = {
        "hidden_states": nc.dram_tensor(
            "hidden_states", [B_LOC, S, E], F32, kind="ExternalInput").ap(),
        "W_attn": nc.dram_tensor("W_attn", [E, 3 * E], F32,
                                 kind="ExternalInput").ap(),
        "b_attn": nc.dram_tensor("b_attn", [3 * E], F32,
                                 kind="ExternalInput").ap(),
        "W_proj": nc.dram_tensor("W_proj", [E, E], F32,
                                 kind="ExternalInput").ap(),
        "b_proj": nc.dram_tensor("b_proj", [E], F32, kind="ExternalInput").ap(),
    }
    outs = {
        "out": nc.dram_tensor("out", [B_LOC, S, E], F32,
                              kind="ExternalOutput").ap(),
    }
    with tile.TileContext(nc) as tc:
        emit(tc, outs, ins)
    nc.compile()
    return nc


_CACHED_NC = None


def kernel(hidden_states, W_attn, b_attn, W_proj, b_proj, trace=False):
    global _CACHED_NC
    from concourse.bass_utils import run_bass_kernel_spmd

    if _CACHED_NC is None:
        _CACHED_NC = build()
    nc = _CACHED_NC

    hidden_states = np.ascontiguousarray(hidden_states, dtype=np.float32)
    W_attn = np.ascontiguousarray(W_attn, dtype=np.float32)
    b_attn = np.ascontiguousarray(b_attn, dtype=np.float32)
    W_proj = np.ascontiguousarray(W_proj, dtype=np.float32)
    b_proj = np.ascontiguousarray(b_proj, dtype=np.float32)

    in_maps = []
    for c in range(NCORES):
        in_maps.append({
            "hidden_states": hidden_states[c * B_LOC:(c + 1) * B_LOC],
            "W_attn": W_attn, "b_attn": b_attn,
            "W_proj": W_proj, "b_proj": b_proj,
        })
    res = run_bass_kernel_spmd(nc, in_maps, core_ids=list(range(NCORES)),
                               trace=trace)
    out = np.concatenate([res.results[c]["out"] for c in range(NCORES)], axis=0)
    kernel.last_result = res
    return out


# revision 8
# speedup vs baseline: 1.7482x; 1.1512x over previous
"""GPT-2 attention (B=16, S=1024, E=768, H=12, D=64) on 8 TRN2 NeuronCores.

Sharding: data-parallel over batch — each core processes B_LOC=2 batch
elements with fully replicated weights. No collectives.

v2 design (vs v1 baseline):
  * all matmul operands fp16 (1 cycle/row at any moving size; PSUM accum f32)
  * attn@v emitted SEQ-major: out[q, d] with a fused ones-column in v giving
    the softmax denominator as PSUM COLUMN 64 -> normalize is a per-partition
    reciprocal [128,1] + tensor_scalar, killing v1's [1,512] reciprocals
    (3.3us each), PE broadcast matmuls and [64,512] normalize muls
  * scores bands accumulate into 2-bank PSUM tiles; ONE wide exp per band
  * causal tri-mask multiply on the (otherwise idle) GpSimd engine
  * x^T via f32r PE transposes (1.5 c/row), copied out as fp16
  * attn out transposed back (fp16 transposes, 1 c/row) for the proj matmul
"""

import sys

sys.path.insert(0, "/opt/trn_rl_repo")

from contextlib import ExitStack

import numpy as np

import concourse.bass as bass
import concourse.mybir as mybir
import concourse.tile as tile
from concourse.masks import make_identity

F32 = mybir.dt.float32
F32R = mybir.dt.float32r
F16 = mybir.dt.float16
AF = mybir.ActivationFunctionType
ALU = mybir.AluOpType

B, S, E = 16, 1024, 768
H, D = 12, 64
NCORES = 8
B_LOC = B // NCORES          # 2 batch elements per core
KC = E // 128                # 6 contraction chunks
ST = S // 128                # 8 seq tiles
PAIRS = H // 2               # 6 head pairs (2 heads per 128-row feature tile)

# band kb covers k in [kb*128, (kb+1)*128), q in [kb*128, S): width 1024-kb*128
BAND_W = [S - kb * 128 for kb in range(ST)]
BAND_OFF = [sum(BAND_W[:kb]) for kb in range(ST)]
EXP_COLS = sum(BAND_W)       # 4608


def emit(tc, outs, ins):
    nc = tc.nc
    x, wa, ba, wp, bp = (ins["hidden_states"], ins["W_attn"], ins["b_attn"],
                         ins["W_proj"], ins["b_proj"])
    out = outs["out"]
    xr = x.bitcast(F32R)
    ba_r = ba.bitcast(F32R)
    bp_r_d = bp.bitcast(F32R)

    ctx = ExitStack()
    with ctx:
        wpool = ctx.enter_context(tc.tile_pool(name="wpool", bufs=1))
        work = ctx.enter_context(tc.tile_pool(name="work", bufs=1))
        ps = ctx.enter_context(tc.tile_pool(name="ps", bufs=1, space="PSUM"))

        # ---------- weights: DMA f32 staging -> fp16 SBUF
        wa_h = []
        for k in range(KC):
            w = wpool.tile([128, 3 * E], F16, tag=f"wa{k}", name=f"wa{k}")
            stage = work.tile([128, 3 * E], F32, tag="wstage", bufs=2,
                              name=f"was{k}")
            nc.sync.dma_start(stage, wa[k * 128:(k + 1) * 128, :])
            nc.scalar.activation(w, stage, AF.Copy)
            wa_h.append(w)
        wp_h = []
        for k in range(KC):
            w = wpool.tile([128, E], F16, tag=f"wp{k}", name=f"wp{k}")
            stage = work.tile([128, E], F32, tag="wpstage", bufs=2,
                              name=f"wps{k}")
            nc.sync.dma_start(stage, wp[k * 128:(k + 1) * 128, :])
            nc.vector.tensor_copy(w, stage)
            wp_h.append(w)

        # q/k bias, feature-major [128, 12]: (p, m) = b_attn[m*128 + p]
        ba_qk = wpool.tile([128, 2 * KC], F32)
        nc.sync.dma_start(ba_qk.bitcast(F32R),
                          ba_r[0:2 * E].rearrange("(m p) -> p m", p=128))
        # v bias and proj bias as rows (outer-product rhs), f32r
        ba_v = wpool.tile([1, E], F32R)
        nc.sync.dma_start(ba_v, ba_r[2 * E:3 * E].unsqueeze(0))
        bp_row = wpool.tile([1, E], F32R)
        nc.sync.dma_start(bp_row, bp_r_d.unsqueeze(0))

        identity = wpool.tile([128, 128], F32)
        make_identity(nc, identity)
        ident_h = wpool.tile([128, 128], F16)
        nc.vector.tensor_copy(ident_h, identity)
        ident_r = wpool.tile([128, 128], F32R)
        nc.vector.tensor_copy(ident_r, identity)

        ones_row32 = wpool.tile([1, 128], F32)
        nc.vector.memset(ones_row32, 1.0)
        ones_row = wpool.tile([1, 128], F32R)
        nc.vector.tensor_copy(ones_row, ones_row32)

        # broadcast v/proj biases to [128, E] (bias-add fuses into PSUM->SBUF)
        biasv_bc = wpool.tile([128, E], F32)
        biasp_bc = wpool.tile([128, E], F32)
        for bc_dst, brow in ((biasv_bc, ba_v), (biasp_bc, bp_row)):
            for n0, nw in ((0, 512), (512, 256)):
                bps = ps.tile([128, 512], F32, tag="avtr", bufs=2,
                              name=f"bbc{n0}_{brow.name}")
                nc.tensor.matmul(bps[:, 0:nw], ones_row, brow[0:1, n0:n0 + nw],
                                 start=True, stop=True)
                nc.scalar.activation(bc_dst[:, n0:n0 + nw], bps[:, 0:nw],
                                     AF.Copy)

        # causal mask via PE: M = UTs.T @ negI accumulated into the scores
        # PSUM before the k.q matmul lands: M[r, c] = -250 if c < r else 0.
        # UTs[p, c] = 1 if p < c (strict upper); negI = -250 * I.
        uts32 = wpool.tile([128, 128], F32)
        nc.gpsimd.memset(uts32, 1.0)
        nc.gpsimd.affine_select(
            out=uts32, in_=uts32, compare_op=ALU.is_gt,
            fill=0.0, base=0, pattern=[[1, 128]], channel_multiplier=-1,
        )
        uts_h = wpool.tile([128, 128], F16)
        nc.vector.tensor_copy(uts_h, uts32)
        negident_h = wpool.tile([128, 128], F16)
        nc.scalar.activation(negident_h, identity, AF.Copy, scale=-250.0)

        for b in range(B_LOC):
            # ---------- A: x^T (fp16) via f32r PE transposes
            xT = []
            for k in range(KC):
                t_ = work.tile([128, S], F16, tag=f"xt{k}", bufs=2,
                               name=f"xT{k}_{b}")
                xT.append(t_)
            for st in range(ST):
                xin = work.tile([128, E], F32R, tag="xin", bufs=3,
                                name=f"xin{b}_{st}")
                nc.sync.dma_start(xin, xr[b, st * 128:(st + 1) * 128, :])
                for k in range(KC):
                    tr_ps = ps.tile([128, 128], F32R, tag="avtr", bufs=2,
                                    name=f"tr{b}_{k}_{st}")
                    nc.tensor.transpose(tr_ps, xin[:, k * 128:(k + 1) * 128],
                                        ident_r)
                    nc.vector.tensor_copy(
                        xT[k][:, st * 128:(st + 1) * 128], tr_ps.bitcast(F32))

            # ---------- B: v seq-major [128, 12, 65] fp16 with ones col
            v_r = []
            for st in range(ST):
                vt = work.tile([128, H, D + 1], F16, tag=f"v{st}",
                               name=f"v{st}_{b}")
                v_r.append(vt)
                nc.vector.memset(vt[:, :, D:D + 1], 1.0)
                acc = ps.tile([128, E], F32, tag="acc", bufs=1,
                              name=f"vacc{b}_{st}")
                for n0, nw in ((0, 512), (512, 256)):
                    for k in range(KC):
                        nc.tensor.matmul(
                            acc[:, n0:n0 + nw],
                            xT[k][:, st * 128:(st + 1) * 128],
                            wa_h[k][:, 2 * E + n0:2 * E + n0 + nw],
                            start=(k == 0), stop=(k == KC - 1))
                nc.vector.tensor_add(
                    vt[:, :, 0:D],
                    acc.rearrange("p (h d) -> p h d", d=D),
                    biasv_bc.rearrange("p (h d) -> p h d", d=D))

            # ---------- C: head pairs
            ao = []
            for st in range(ST):
                at = work.tile([128, E], F16, tag=f"ao{st}", name=f"ao{st}_{b}")
                ao.append(at)
            for g in range(PAIRS // 2):
                # 4-head group: C1 + scores/exp for 2 pairs, then batched av
                exps = []
                for tt in range(2):
                    t = 2 * g + tt
                    q_r = work.tile([128, S], F16, tag="qt", bufs=3,
                                    name=f"q{t}_{b}")
                    k_r = work.tile([128, S], F16, tag="kt", bufs=3,
                                    name=f"k{t}_{b}")
                    for dst, m in ((q_r, t), (k_r, KC + t)):
                        qkacc = ps.tile([128, S], F32, tag="acc", bufs=1,
                                        name=f"qk{b}_{m}")
                        for c0 in (0, 512):
                            for k in range(KC):
                                nc.tensor.matmul(
                                    qkacc[:, c0:c0 + 512],
                                    wa_h[k][:, m * 128:(m + 1) * 128],
                                    xT[k][:, c0:c0 + 512],
                                    start=(k == 0), stop=(k == KC - 1))
                        nc.scalar.activation(dst, qkacc, AF.Identity,
                                             bias=ba_qk[:, m:m + 1])

                    for hh in range(2):
                        h = 2 * t + hh
                        po = hh * 64
                        exp_h = work.tile([128, EXP_COLS], F16, tag="exp",
                                          bufs=5, name=f"exp{b}_{h}")
                        exps.append(exp_h)
                        for kb in range(ST):
                            k0 = kb * 128
                            w = BAND_W[kb]
                            off = BAND_OFF[kb]
                            sc = ps.tile([128, 1024], F32, tag="sc", bufs=2,
                                         name=f"sc{b}_{h}_{kb}")
                            # causal mask for the diag block, on PE (bank 0
                            # group: mask -> first chunk; bank 1 its own)
                            nc.tensor.matmul(sc[:, 0:128], uts_h, negident_h,
                                             start=True, stop=False)
                            c0 = k0
                            while c0 < S:
                                cw = min(512, S - c0)
                                lo = c0 - k0
                                nc.tensor.matmul(
                                    sc[:, lo:lo + cw],
                                    k_r[po:po + 64, k0:k0 + 128],
                                    q_r[po:po + 64, c0:c0 + cw],
                                    start=(lo >= 512), stop=True)
                                c0 += cw
                            nc.scalar.activation(
                                exp_h[:, off:off + w], sc[:, 0:w],
                                AF.Exp, scale=0.125)

                for qt in range(ST):
                    av4 = ps.tile([128, 4, D + 1], F32, tag="avtr", bufs=2,
                                  name=f"av{b}_{g}_{qt}")
                    for hi in range(4):
                        for kb in range(qt + 1):
                            sl = BAND_OFF[kb] + (qt - kb) * 128
                            nc.tensor.matmul(
                                av4[:, hi, :], exps[hi][:, sl:sl + 128],
                                v_r[kb][:, 4 * g + hi, :],
                                start=(hi == 0 and kb == 0),
                                stop=(hi == 3 and kb == qt))
                    rc4 = work.tile([128, 4, 1], F32, tag="rc", bufs=4,
                                    name=f"rc{b}_{g}_{qt}")
                    nc.vector.reciprocal(rc4, av4[:, :, D:D + 1])
                    nc.vector.tensor_mul(
                        ao[qt][:, g * 256:(g + 1) * 256].rearrange(
                            "p (h d) -> p h d", d=D),
                        av4[:, :, 0:D],
                        rc4.broadcast_to((128, 4, D)))

            # ---------- transpose attn out back to feature-major
            aoT = []
            for k in range(KC):
                at = work.tile([128, S], F16, tag=f"aot{k}", name=f"aoT{k}_{b}")
                aoT.append(at)
            for st in range(ST):
                for k in range(KC):
                    tr_ps = ps.tile([128, 128], F16, tag="avtr", bufs=2,
                                    name=f"aotr{b}_{st}_{k}")
                    nc.tensor.transpose(
                        tr_ps, ao[st][:, k * 128:(k + 1) * 128], ident_h)
                    nc.vector.tensor_copy(
                        aoT[k][:, st * 128:(st + 1) * 128], tr_ps)

            # ---------- D: proj
            for st in range(ST):
                outt = work.tile([128, E], F32, tag="outt", bufs=2,
                                 name=f"outt{b}_{st}")
                pacc = ps.tile([128, E], F32, tag="acc", bufs=1,
                               name=f"pacc{b}_{st}")
                for n0, nw in ((0, 512), (512, 256)):
                    for k in range(KC):
                        nc.tensor.matmul(
                            pacc[:, n0:n0 + nw],
                            aoT[k][:, st * 128:(st + 1) * 128],
                            wp_h[k][:, n0:n0 + nw],
                            start=(k == 0), stop=(k == KC - 1))
                nc.vector.tensor_add(outt, pacc, biasp_bc)
                nc.sync.dma_start(out[b, st * 128:(st + 1) * 128, :], outt)


def build():
    from concourse import bacc

    nc = bacc.Bacc("TRN2", target_bir_lowering=False, debug=False)
    ins = {
        "hidden_states": nc.dram_tensor(
            "hidden_states", [B_LOC, S, E], F32, kind="ExternalInput").ap(),
        "W_attn": nc.dram_tensor("W_attn", [E, 3 * E], F32,
                                 kind="ExternalInput").ap(),
        "b_attn": nc.dram_tensor("b_attn", [3 * E], F32,
                                 kind="ExternalInput").ap(),
        "W_proj": nc.dram_tensor("W_proj", [E, E], F32,
                                 kind="ExternalInput").ap(),
        "b_proj": nc.dram_tensor("b_proj", [E], F32, kind="ExternalInput").ap(),
    }
    outs = {
        "out": nc.dram_tensor("out", [B_LOC, S, E], F32,
                              kind="ExternalOutput").ap(),
    }
    with tile.TileContext(nc) as tc:
        emit(tc, outs, ins)
    nc.compile()
    return nc


_CACHED_NC = None


def kernel(hidden_states, W_attn, b_attn, W_proj, b_proj, trace=False):
    global _CACHED_NC
    from concourse.bass_utils import run_bass_kernel_spmd

    if _CACHED_NC is None:
        _CACHED_NC = build()
    nc = _CACHED_NC

    hidden_states = np.ascontiguousarray(hidden_states, dtype=np.float32)
    W_attn = np.ascontiguousarray(W_attn, dtype=np.float32)
    b_attn = np.ascontiguousarray(b_attn, dtype=np.float32)
    W_proj = np.ascontiguousarray(W_proj, dtype=np.float32)
    b_proj = np.ascontiguousarray(b_proj, dtype=np.float32)

    in_maps = []
    for c in range(NCORES):
        in_maps.append({
            "hidden_states": hidden_states[c * B_LOC:(c + 1) * B_LOC],
            "W_attn": W_attn, "b_attn": b_attn,
            "W_proj": W_proj, "b_proj": b_proj,
        })
    res = run_bass_kernel_spmd(nc, in_maps, core_ids=list(range(NCORES)),
                               trace=trace)
    out = np.concatenate([res.results[c]["out"] for c in range(NCORES)], axis=0)
    kernel.last_result = res
    return out
